# revision 18
# baseline (speedup 1.0000x reference)
"""Trainium2 Bass kernel for a 2-layer LSTM (64, 32) + MLP head.

Model (PyTorch semantics, eval mode):
    h1 = LSTM(4 -> 64)(x)            x: [B=4096, T=512, 4]
    h2 = LSTM(64 -> 32)(h1)
    y  = (relu(h2[:, -1] @ w_fc1.T + b_fc1)) @ w_fc2.T + b_fc2   # [B, 1]

Sharding: data-parallel over batch across 8 NeuronCores (512 rows each),
weights replicated. Inside each core the state is kept *transposed*
([units, batch]) so the per-timestep recurrent matmuls have batch on the
moving free dimension (N=512) and the gate nonlinearities run as a few
wide ops on full 96-partition stacks (layer-1 and layer-2 gates stacked).

State tile S [97, 512]: rows 0:64 = h1^T, rows 64:96 = h2^T, row 96 =
ones (bias row).  Both layers' recurrent matmuls use rhs S[0:97] (base
partition 0 — the PE moving operand must start at 0 to span >32
partitions); layer-1's weight rows over the h2 region are zeros, and
layer-2's over nothing (it genuinely uses h1+h2).  Biases ride the
ones-row through the matmul (incl. the fc1 bias in the head).
The input projection is a separate K=4 matmul per gate accumulating into
the same PSUM bank; x_t arrives per step by DMA into a small [4, 512]
rotating tile (x is recurrence-independent, so these prefetch ahead).

PSUM gate tile P [96, 2048] (4 banks): free slices i@0, f@512, o@1024,
g@1536; partitions 0:64 = layer-1 gate, 64:96 = layer-2 gate.  Sigmoid
is then ONE activation op over [96, 1536] (i,f,o) and tanh one op over
[96, 512] (g); the cell/hidden updates are [96, 512] vector ops.
"""

import numpy as np
from contextlib import ExitStack

import concourse.bass as bass
import concourse.tile as tile
from concourse import bacc, mybir
from concourse import bass_utils

AF = mybir.ActivationFunctionType

B, T, D_IN, H1, H2 = 4096, 512, 4, 64, 32
NCORES = 8
BL = B // NCORES  # 512 batch rows per core

F32 = mybir.dt.float32
# Compute dtypes (flip for perf/accuracy trades):
DT = mybir.dt.bfloat16  # weights / state / gate-activation dtype
CDT = mybir.dt.bfloat16  # cell-state dtype

HS = H1 + H2  # 96: stacked (layer1, layer2) partition extent


def _build(n_steps: int = T):
    """Build the SPMD single-core Bass program (same NEFF on all 8 cores)."""
    nc = bacc.Bacc("TRN2", target_bir_lowering=False, debug=False)

    xT = nc.dram_tensor("xT", [n_steps * 4, BL], DT, kind="ExternalInput")
    w12e = nc.dram_tensor("w12e", [105, 4 * HS], DT, kind="ExternalInput")
    w12o = nc.dram_tensor("w12o", [105, 4 * HS], DT, kind="ExternalInput")
    wf1 = nc.dram_tensor("wf1", [97, 16], DT, kind="ExternalInput")
    wf2 = nc.dram_tensor("wf2", [16, 1], DT, kind="ExternalInput")
    bf2 = nc.dram_tensor("bf2", [1, 1], F32, kind="ExternalInput")
    out = nc.dram_tensor("out", [1, BL], F32, kind="ExternalOutput")

    with tile.TileContext(nc) as tc, ExitStack() as ctx:
        const = ctx.enter_context(tc.tile_pool(name="const", bufs=1))
        xpool = ctx.enter_context(tc.tile_pool(name="xp", bufs=8))
        gates = ctx.enter_context(tc.tile_pool(name="gates", bufs=3))

        W12E = const.tile([105, 4 * HS], DT, tag="W12E")
        nc.sync.dma_start(W12E[:], w12e.ap())
        W12O = const.tile([105, 4 * HS], DT, tag="W12O")
        nc.sync.dma_start(W12O[:], w12o.ap())
        WF1 = const.tile([97, 16], DT, tag="WF1")
        nc.sync.dma_start(WF1[:], wf1.ap())
        WF2 = const.tile([16, 1], DT, tag="WF2")
        nc.sync.dma_start(WF2[:], wf2.ap())
        BF2 = const.tile([1, 1], F32, tag="BF2")
        nc.sync.dma_start(BF2[:], bf2.ap())

        S = const.tile([105, BL], DT, tag="S")
        C = const.tile([HS, BL], CDT, tag="C")
        nc.vector.memset(S[:], 0.0)
        nc.vector.memset(S[96:97, :], 1.0)
        nc.vector.memset(C[:], 0.0)

        # Per-gate PSUM tiles: Pf [96,512], Pig [96,1024] (i|g adjacent,
        # one merged sigmoid), Po [96,512].  The g-gate weights are scaled
        # by 2 on the host so tanh(z_g) = 2*sigmoid(2 z_g) - 1 comes out of
        # the SAME sigmoid pass as the i gate; the affine fix-up folds into
        # the cell update (scalar_tensor_tensor + an off-chain subtract):
        #   c' = 2*(i*s) + (f*c - i),  s = sigmoid(2 z_g)
        #
        # x_t lives in rows 97:101 (even t) / 101:105 (odd t) of S, DMA'd
        # one step ahead; the recurrent matmul contracts K=105 with weight
        # variants W12E/W12O that carry w_ih1 on the active x rows and
        # zeros on the inactive ones.  (DMA writes have no partition-
        # alignment restriction, unlike compute-engine access patterns.)
        #
        # Software-pipelined over layers: at iteration k the layer-1
        # partition computes LSTM-1 step k while the layer-2 partition
        # computes LSTM-2 step k-1 (both read h1_{k-1} from S).
        # Iteration 0 produces garbage layer-2 state (cleared after);
        # iteration n_steps produces garbage layer-1 state (the head
        # weights are zero over the h1 rows).
        GSEL = {"i": 0, "f": 1, "g": 2, "o": 3}

        def emit_x_dma(step):
            xrow = 97 if step % 2 == 0 else 101
            nc.sync.dma_start(S[xrow : xrow + 4, :],
                              xT.ap()[4 * step : 4 * step + 4, :])

        with tc.tile_pool(name="psum", bufs=2, space="PSUM") as psum:
            emit_x_dma(0)
            for k in range(n_steps + 1):
                W12 = W12E if k % 2 == 0 else W12O
                Pf = psum.tile([HS, BL], F32, tag="Pf")
                Pig = psum.tile([HS, 2 * BL], F32, tag="Pig")
                Po = psum.tile([HS, BL], F32, tag="Po")
                for gate, dest in (
                    ("f", Pf[:, :]),
                    ("i", Pig[:, 0:BL]),
                    ("g", Pig[:, BL:]),
                    ("o", Po[:, :]),
                ):
                    gsel = GSEL[gate]
                    nc.tensor.matmul(
                        dest,
                        W12[:, gsel * HS : (gsel + 1) * HS],
                        S[0:105, :],
                        start=True,
                        stop=True,
                    )
                if k + 1 < n_steps:
                    emit_x_dma(k + 1)

                SIGF = gates.tile([HS, BL], DT, tag="SIGF")
                SIGIG = gates.tile([HS, 2 * BL], DT, tag="SIGIG")
                SIGO = gates.tile([HS, BL], DT, tag="SIGO")
                nc.scalar.activation(SIGF[:], Pf[:, :], AF.Sigmoid)
                nc.scalar.activation(SIGIG[:], Pig[:, :], AF.Sigmoid)
                nc.scalar.activation(SIGO[:], Po[:, :], AF.Sigmoid)

                V1 = gates.tile([HS, BL], CDT, tag="V1")
                V2 = gates.tile([HS, BL], CDT, tag="V2")
                U0 = gates.tile([HS, BL], DT, tag="U0")
                nc.vector.tensor_mul(V1[:], SIGF[:], C[:])                # f*c
                nc.vector.tensor_mul(U0[:], SIGIG[:, 0:BL], SIGIG[:, BL:])  # i*s
                nc.vector.tensor_sub(V2[:], V1[:], SIGIG[:, 0:BL])       # f*c - i
                nc.vector.scalar_tensor_tensor(                          # c' = 2*i*s + (f*c - i)
                    C[:], U0[:], 2.0, V2[:],
                    mybir.AluOpType.mult, mybir.AluOpType.add,
                )
                TC = gates.tile([HS, BL], DT, tag="TC")
                nc.scalar.activation(TC[:], C[:], AF.Tanh)
                nc.vector.tensor_mul(S[0:HS, :], SIGO[:], TC[:])         # h
                if k == 0:
                    # wipe the garbage layer-2 state from the pipeline warmup
                    nc.vector.memset(S[H1:HS, :], 0.0)
                    nc.vector.memset(C[H1:HS, :], 0.0)

        # MLP head on h2 at the last timestep (rows 64:96 of S).
        with tc.tile_pool(name="psum_head", bufs=1, space="PSUM") as psh:
            PF = psh.tile([16, BL], F32, tag="PF")
            nc.tensor.matmul(PF[:], WF1[:, :], S[0:97, :], start=True, stop=True)
            Z = gates.tile([16, BL], DT, tag="Z")
            nc.scalar.activation(Z[:], PF[:], AF.Relu)
            PO = psh.tile([1, BL], F32, tag="PO")
            nc.tensor.matmul(PO[:], WF2[:, :], Z[:], start=True, stop=True)
            Y = gates.tile([1, BL], F32, tag="Y")
            nc.scalar.activation(Y[:], PO[:], AF.Identity, bias=BF2[:, 0:1])
            nc.sync.dma_start(out.ap(), Y[:])

    nc.compile()
    return nc


def _pack_weights(inputs, np_dt):
    w_ih1, w_hh1 = inputs["w_ih1"], inputs["w_hh1"]
    w_ih2, w_hh2 = inputs["w_ih2"], inputs["w_hh2"]
    b1 = (inputs["b_ih1"] + inputs["b_hh1"]).astype(np.float32)
    b2 = (inputs["b_ih2"] + inputs["b_hh2"]).astype(np.float32)

    def variant(parity):
        # rows 0:64 h1, 64:96 h2, 96 ones, 97:101 x (even), 101:105 x (odd)
        xz = np.zeros((4 * H1, 4), np.float32)
        xe, xo = (w_ih1, xz) if parity == 0 else (xz, w_ih1)
        # layer-1 block [105, 256]
        w1 = np.concatenate(
            [w_hh1, np.zeros((4 * H1, 32), np.float32), b1[:, None], xe, xo],
            axis=1).T
        # layer-2 block [105, 128] (x rows always zero)
        w2 = np.concatenate(
            [w_ih2, w_hh2, b2[:, None], np.zeros((4 * H2, 8), np.float32)],
            axis=1).T
        w = np.concatenate(
            [np.concatenate([w1[:, g * H1 : (g + 1) * H1],
                             w2[:, g * H2 : (g + 1) * H2]], axis=1)
             for g in range(4)], axis=1)
        # tanh-as-sigmoid: scale the g-gate block (cols 192:288) by 2
        w[:, 2 * HS : 3 * HS] *= 2.0
        return np.ascontiguousarray(w).astype(np_dt)

    return {
        "w12e": variant(0),
        "w12o": variant(1),
        "wf1": np.ascontiguousarray(np.concatenate(
            [np.zeros((64, 16), np.float32), inputs["w_fc1"].T,
             inputs["b_fc1"][None, :]], axis=0)).astype(np_dt),
        "wf2": np.ascontiguousarray(inputs["w_fc2"].T).astype(np_dt),
        "bf2": np.ascontiguousarray(inputs["b_fc2"][:, None]).astype(np.float32),
    }


_built = {}


def _get_nc(n_steps):
    if n_steps not in _built:
        _built[n_steps] = _build(n_steps)
    return _built[n_steps]


def _run(inputs, n_steps=T, **run_kwargs):
    np_dt = mybir.dt.np(DT)
    x = np.asarray(inputs["x"], np.float32)
    nb = x.shape[0]
    ncores = NCORES
    bl = nb // ncores
    assert bl == BL and x.shape[1] >= n_steps
    shared = _pack_weights({k: np.asarray(v, np.float32) for k, v in inputs.items()
                            if k != "x"} | {}, np_dt)
    in_maps = []
    for c in range(ncores):
        xs = x[c * bl : (c + 1) * bl, :n_steps, :]  # [BL, T, 4]
        xT = np.ascontiguousarray(xs.transpose(1, 2, 0).reshape(n_steps * 4, bl))
        in_maps.append(dict(shared, xT=xT.astype(np_dt)))
    nc = _get_nc(n_steps)
    res = bass_utils.run_bass_kernel_spmd(
        nc, in_maps, core_ids=list(range(ncores)), **run_kwargs
    )
    y = np.concatenate(
        [np.asarray(r["out"], np.float32).reshape(bl, 1) for r in res.results], axis=0
    )
    return y, res


def kernel(**inputs) -> np.ndarray:
    y, _ = _run(inputs)
    return y


# revision 20
# speedup vs baseline: 1.1668x; 1.1668x over previous
"""Trainium2 Bass kernel for a 2-layer LSTM (64, 32) + MLP head.

Model (PyTorch semantics, eval mode):
    h1 = LSTM(4 -> 64)(x)            x: [B=4096, T=512, 4]
    h2 = LSTM(64 -> 32)(h1)
    y  = (relu(h2[:, -1] @ w_fc1.T + b_fc1)) @ w_fc2.T + b_fc2   # [B, 1]

Sharding: data-parallel over batch across 8 NeuronCores (512 rows each),
weights replicated. Inside each core the state is kept *transposed*
([units, batch]) so the per-timestep recurrent matmuls have batch on the
moving free dimension (N=512) and the gate nonlinearities run as a few
wide ops on full 96-partition stacks (layer-1 and layer-2 gates stacked).

State tile S [97, 512]: rows 0:64 = h1^T, rows 64:96 = h2^T, row 96 =
ones (bias row).  Both layers' recurrent matmuls use rhs S[0:97] (base
partition 0 — the PE moving operand must start at 0 to span >32
partitions); layer-1's weight rows over the h2 region are zeros, and
layer-2's over nothing (it genuinely uses h1+h2).  Biases ride the
ones-row through the matmul (incl. the fc1 bias in the head).
The input projection is a K=4 matmul per gate accumulating into the
same PSUM bank; x_t arrives per step by DMA into a small [4, 512]
rotating tile (x is recurrence-independent, so these prefetch ahead and
the matmuls run one step early, filling the TensorE pipe during the
previous step's ACT/DVE chain).

Each gate gets its own PSUM tile (per-tile dependency tracking lets each
sigmoid start as soon as its own gate's matmuls finish); layer-1 and
layer-2 are fused into one M=96 matmul per gate (both contract the same
rhs S[0:97]).  Gate order f,i,g,o: sigmoid(f) (and f*c) overlap the
remaining matmuls; sigmoid(o) fills the ACT gap while the vector engine
runs the cell update; tanh(c) and h close the serial chain.
"""

import numpy as np
from contextlib import ExitStack

import concourse.bass as bass
import concourse.tile as tile
from concourse import bacc, mybir
from concourse import bass_utils

AF = mybir.ActivationFunctionType

B, T, D_IN, H1, H2 = 4096, 512, 4, 64, 32
NCORES = 8
BL = B // NCORES  # 512 batch rows per core

F32 = mybir.dt.float32
# Compute dtypes (flip for perf/accuracy trades):
DT = mybir.dt.bfloat16  # weights / state / gate-activation dtype
CDT = mybir.dt.bfloat16  # cell-state dtype

HS = H1 + H2  # 96: stacked (layer1, layer2) partition extent


def _build(n_steps: int = T):
    """Build the SPMD single-core Bass program (same NEFF on all 8 cores)."""
    nc = bacc.Bacc("TRN2", target_bir_lowering=False, debug=False)

    xT = nc.dram_tensor("xT", [n_steps * 4, BL], DT, kind="ExternalInput")
    w12t = nc.dram_tensor("w12t", [97, 4 * HS], DT, kind="ExternalInput")
    w1x = nc.dram_tensor("w1x", [4, 4 * HS], DT, kind="ExternalInput")
    wf1 = nc.dram_tensor("wf1", [97, 16], DT, kind="ExternalInput")
    wf2 = nc.dram_tensor("wf2", [16, 1], DT, kind="ExternalInput")
    bf2 = nc.dram_tensor("bf2", [1, 1], F32, kind="ExternalInput")
    out = nc.dram_tensor("out", [1, BL], F32, kind="ExternalOutput")

    with tile.TileContext(nc) as tc, ExitStack() as ctx:
        const = ctx.enter_context(tc.tile_pool(name="const", bufs=1))
        xpool = ctx.enter_context(tc.tile_pool(name="xp", bufs=8))
        gates = ctx.enter_context(tc.tile_pool(name="gates", bufs=3))

        W12 = const.tile([97, 4 * HS], DT, tag="W12")
        nc.sync.dma_start(W12[:], w12t.ap())
        W1X = const.tile([4, 4 * HS], DT, tag="W1X")
        nc.sync.dma_start(W1X[:], w1x.ap())
        WF1 = const.tile([97, 16], DT, tag="WF1")
        nc.sync.dma_start(WF1[:], wf1.ap())
        WF2 = const.tile([16, 1], DT, tag="WF2")
        nc.sync.dma_start(WF2[:], wf2.ap())
        BF2 = const.tile([1, 1], F32, tag="BF2")
        nc.sync.dma_start(BF2[:], bf2.ap())

        S = const.tile([97, BL], DT, tag="S")
        C = const.tile([HS, BL], CDT, tag="C")
        nc.vector.memset(S[:], 0.0)
        nc.vector.memset(S[96:97, :], 1.0)
        nc.vector.memset(C[:], 0.0)

        # Per-gate PSUM tiles (per-bank dependency tracking, so each
        # activation op starts as soon as its own gate's matmuls finish):
        # Pf [96,512] (f), Pio [96,1024] (i|o), Pg [96,512] (g).
        # Layer-1 (cols 0:64 of each gate's weight block) and layer-2
        # (cols 64:96) are fused into ONE M=96 matmul per gate — they
        # share the rhs S[0:97].  The x-projection is a K=4 matmul per
        # gate (M=96, layer-2 columns zero) emitted one step AHEAD
        # (start=True), so it fills the TensorE pipe during the previous
        # step's ACT/DVE chain; the recurrent matmul accumulates on top.
        #
        # Software-pipelined over layers: at iteration k the layer-1
        # partition computes LSTM-1 step k while the layer-2 partition
        # computes LSTM-2 step k-1 (both read h1_{k-1} from S).
        # Iteration 0 produces garbage layer-2 state (cleared after);
        # iteration n_steps produces garbage layer-1 state (the head
        # weights are zero over the h1 rows).
        GSEL = {"i": 0, "f": 1, "g": 2, "o": 3}

        def alloc_P():
            Pf = psum.tile([HS, BL], F32, tag="Pf")
            Pi = psum.tile([HS, BL], F32, tag="Pi")
            Pg = psum.tile([HS, BL], F32, tag="Pg")
            Po = psum.tile([HS, BL], F32, tag="Po")
            # (gate, dest-ap) in emission order: f, i, g, o —
            # f first (feeds f*c as early as possible), o last (only
            # needed at the very end for h = o * tanh(c)).
            return [
                ("f", Pf[:, :]),
                ("i", Pi[:, :]),
                ("g", Pg[:, :]),
                ("o", Po[:, :]),
            ], Pf, Pi, Pg, Po

        def emit_x_mms(banks, step):
            XTT = xpool.tile([4, BL], DT, tag="xt")
            nc.sync.dma_start(XTT[:], xT.ap()[4 * step : 4 * step + 4, :])
            for gate, dest in banks:
                gsel = GSEL[gate]
                nc.tensor.matmul(
                    dest,
                    W1X[:, gsel * HS : (gsel + 1) * HS],
                    XTT[:],
                    start=True,
                    stop=False,
                )

        with tc.tile_pool(name="psum", bufs=2, space="PSUM") as psum:
            banks, Pf, Pi, Pg, Po = alloc_P()
            emit_x_mms(banks, 0)
            for k in range(n_steps + 1):
                has_x = k < n_steps  # P already holds the x contribution
                for gate, dest in banks:
                    gsel = GSEL[gate]
                    nc.tensor.matmul(
                        dest,
                        W12[:, gsel * HS : (gsel + 1) * HS],
                        S[0:97, :],
                        start=not has_x,
                        stop=True,
                    )

                if k + 1 <= n_steps:
                    nbanks, nPf, nPi, nPg, nPo = alloc_P()
                    if k + 1 < n_steps:
                        emit_x_mms(nbanks, k + 1)

                SIGF = gates.tile([HS, BL], DT, tag="SIGF")
                SIGI = gates.tile([HS, BL], DT, tag="SIGI")
                G = gates.tile([HS, BL], DT, tag="G")
                SIGO = gates.tile([HS, BL], DT, tag="SIGO")
                nc.scalar.activation(SIGF[:], Pf[:, :], AF.Sigmoid)
                nc.scalar.activation(SIGI[:], Pi[:, :], AF.Sigmoid)
                nc.scalar.activation(G[:], Pg[:, :], AF.Tanh)
                nc.scalar.activation(SIGO[:], Po[:, :], AF.Sigmoid)

                U = gates.tile([HS, BL], DT, tag="U")
                V = gates.tile([HS, BL], CDT, tag="V")
                nc.vector.tensor_mul(V[:], SIGF[:], C[:])               # f*c
                nc.vector.tensor_mul(U[:], SIGI[:], G[:])               # i*g
                nc.vector.tensor_add(C[:], U[:], V[:])                  # c'
                TC = gates.tile([HS, BL], DT, tag="TC")
                nc.scalar.activation(TC[:], C[:], AF.Tanh)
                nc.vector.tensor_mul(S[0:HS, :], SIGO[:], TC[:])        # h
                if k == 0:
                    # wipe the garbage layer-2 state from the pipeline warmup
                    nc.vector.memset(S[H1:HS, :], 0.0)
                    nc.vector.memset(C[H1:HS, :], 0.0)
                if k + 1 <= n_steps:
                    banks, Pf, Pi, Pg, Po = nbanks, nPf, nPi, nPg, nPo

        # MLP head on h2 at the last timestep (rows 64:96 of S).
        with tc.tile_pool(name="psum_head", bufs=1, space="PSUM") as psh:
            PF = psh.tile([16, BL], F32, tag="PF")
            nc.tensor.matmul(PF[:], WF1[:, :], S[0:97, :], start=True, stop=True)
            Z = gates.tile([16, BL], DT, tag="Z")
            nc.scalar.activation(Z[:], PF[:], AF.Relu)
            PO = psh.tile([1, BL], F32, tag="PO")
            nc.tensor.matmul(PO[:], WF2[:, :], Z[:], start=True, stop=True)
            Y = gates.tile([1, BL], F32, tag="Y")
            nc.scalar.activation(Y[:], PO[:], AF.Identity, bias=BF2[:, 0:1])
            nc.sync.dma_start(out.ap(), Y[:])

    nc.compile()
    return nc


def _pack_weights(inputs, np_dt):
    w_ih1, w_hh1 = inputs["w_ih1"], inputs["w_hh1"]
    w_ih2, w_hh2 = inputs["w_ih2"], inputs["w_hh2"]
    b1 = (inputs["b_ih1"] + inputs["b_hh1"]).astype(np.float32)
    b2 = (inputs["b_ih2"] + inputs["b_hh2"]).astype(np.float32)
    # Layer-1 gate weights as [97, 256]: rows = [w_hh1^T(64); zeros(32);
    # bias1(1)] matching rhs S[0:97] = [h1; h2(ignored); ones].
    z32 = np.zeros((4 * H1, 32), np.float32)
    w1t = np.concatenate([w_hh1, z32, b1[:, None]], axis=1).T
    # Layer-2 gate weights as [97, 128]: rows = [w_ih2^T(64); w_hh2^T(32);
    # bias2(1)].
    w2t = np.concatenate([w_ih2, w_hh2, b2[:, None]], axis=1).T
    # Fused per-gate blocks [97, 96]: layer-1 output units in cols 0:64,
    # layer-2 in cols 64:96 (one M=96 matmul per gate).
    w12t = np.concatenate(
        [np.concatenate([w1t[:, g * H1 : (g + 1) * H1],
                         w2t[:, g * H2 : (g + 1) * H2]], axis=1)
         for g in range(4)], axis=1)
    # Input projection [4, 384]: per gate [w_ih1^T (64) | zeros (32)].
    zx = np.zeros((4, H2), np.float32)
    w1x = np.concatenate(
        [np.concatenate([w_ih1.T[:, g * H1 : (g + 1) * H1], zx], axis=1)
         for g in range(4)], axis=1)
    return {
        "w12t": np.ascontiguousarray(w12t).astype(np_dt),
        "w1x": np.ascontiguousarray(w1x).astype(np_dt),
        "wf1": np.ascontiguousarray(np.concatenate(
            [np.zeros((64, 16), np.float32), inputs["w_fc1"].T,
             inputs["b_fc1"][None, :]], axis=0)).astype(np_dt),
        "wf2": np.ascontiguousarray(inputs["w_fc2"].T).astype(np_dt),
        "bf2": np.ascontiguousarray(inputs["b_fc2"][:, None]).astype(np.float32),
    }


_built = {}


def _get_nc(n_steps):
    if n_steps not in _built:
        _built[n_steps] = _build(n_steps)
    return _built[n_steps]


def _run(inputs, n_steps=T, **run_kwargs):
    np_dt = mybir.dt.np(DT)
    x = np.asarray(inputs["x"], np.float32)
    nb = x.shape[0]
    ncores = NCORES
    bl = nb // ncores
    assert bl == BL and x.shape[1] >= n_steps
    shared = _pack_weights({k: np.asarray(v, np.float32) for k, v in inputs.items()
                            if k != "x"} | {}, np_dt)
    in_maps = []
    for c in range(ncores):
        xs = x[c * bl : (c + 1) * bl, :n_steps, :]  # [BL, T, 4]
        xT = np.ascontiguousarray(xs.transpose(1, 2, 0).reshape(n_steps * 4, bl))
        in_maps.append(dict(shared, xT=xT.astype(np_dt)))
    nc = _get_nc(n_steps)
    res = bass_utils.run_bass_kernel_spmd(
        nc, in_maps, core_ids=list(range(ncores)), **run_kwargs
    )
    y = np.concatenate(
        [np.asarray(r["out"], np.float32).reshape(bl, 1) for r in res.results], axis=0
    )
    return y, res


def kernel(**inputs) -> np.ndarray:
    y, _ = _run(inputs)
    return y
